# revision 5
# baseline (speedup 1.0000x reference)
"""CPD reconstruction at observed entries (embedding-lookup style) on 8 TRN2 cores.

rec[n] = sum_r f0[i0[n],r] * f1[i1[n],r] * f2[i2[n],r]   for n in [0, 1M)

Strategy: data-parallel over the nnz axis across the 8 cores (125k entries
each).  The per-row random gathers use the Anthropic extended DMA-gather
instruction (InstDMAGatherAnt): ONE instruction emits 8192 row descriptors
from an int16 index list, generated by a GPSIMD Q7 core pair; rotating the
4 SWDGE queues runs 4 pairs in parallel.  That replaces the baseline's
per-128-row indirect DMAs (994ns fixed SWDGE cost each, serial on the Pool
engine) and is ~4.5x faster end to end.

dma_gather constraints and how they are met:
  * int16 indices (0..32767): entries are processed in chunks of 32768; the
    host builds a per-(chunk, mode) DEDUP TABLE holding np.unique(rows) of
    that chunk (<= 32768 rows, so the rank of each entry's row always fits
    int16), and remaps indices to ranks.  Gathers stay in entry order - no
    realignment anywhere.
  * the HBM row stride must be a multiple of 256B: dedup tables are padded
    to 64 f32 per row.  The payload per descriptor stays 128B (elem_size=32,
    elem_step=64) - the bass-level elem%256B assert is Q7-side unnecessary,
    so we emit the instruction directly (dma_gather_raw).
  * descriptors are packetized (single_packet=False); the default
    single-packet mode caps a gather at 64 descriptors/lane and crashes the
    exec unit beyond that.

Per sub-chunk of 8192 entries the three modes' gathered rows land as
[128, 64, 32] tiles (entry n at partition n%128, column n//128); DVE
multiplies the three tiles and reduces over rank into the output column
block.  Host un-interleaves at the end.
"""

import numpy as np

NNZ = 1_000_000
RANK = 32
N_CORES = 8
N_PER_CORE = NNZ // N_CORES  # 125_000
P = 128
CHUNK = 32_768  # entries per dedup-table chunk (int16 rank guarantee)
NCHUNK = -(-N_PER_CORE // CHUNK)  # 4
MODES = 3
S = 8_192  # indices per dma_gather; uniform size keeps the 4 Q7 pairs balanced
SUB_SIZES = [S] * 15 + [2_176]
N_PAD = sum(SUB_SIZES)  # 125_056 entries per core after padding
SUB_BASE = [0]
for _s in SUB_SIZES:
    SUB_BASE.append(SUB_BASE[-1] + _s)
NSUB = len(SUB_SIZES)
DT_ROWS = CHUNK  # dedup table rows per (chunk, mode)
DT_W = 64  # f32 per dedup row (256B stride; first 32 hold data)
W_OUT = N_PAD // P  # 977 output columns
NQ = 4  # SWDGE queues
# idx16 column offset of (sub-chunk t, mode m); width SUB_SIZES[t]//16
_IDX_OFF = []
_o = 0
for _t in range(NSUB):
    for _m in range(MODES):
        _IDX_OFF.append(_o)
        _o += SUB_SIZES[_t] // 16
IDX_COLS = _o

_cache: dict = {}


def dma_gather_raw(g, out_ap, in_ap, idxs_ap, num_idxs, num_idxs_reg,
                   elem_size, elem_step, queue_num):
    """bass.dma_gather without the elem_size%256B assert (stride must still be
    a 256B multiple; the Q7 kernel handles any payload size) and with
    packetized descriptors."""
    import concourse.mybir as mybir

    g._assert_queue_num(queue_num)
    assert idxs_ap.dtype == mybir.dt.int16
    assert in_ap.dtype == out_ap.dtype
    stride_bytes = elem_step * mybir.dt.size(in_ap.dtype)
    stride_bytes_256 = stride_bytes // 256
    assert stride_bytes % 256 == 0 and 0 < stride_bytes_256 < 256
    assert in_ap.ap[0][0] == elem_step
    assert in_ap.ap[-1][1] == out_ap.ap[-1][1] == elem_size

    _in_ap = g.lower_ap_dma(in_ap, for_custom_bir_dma=True)
    _idxs_ap = g.lower_ap(idxs_ap)
    _out_ap = g.lower_ap(out_ap)
    return g.add_instruction(
        mybir.InstDMAGatherAnt(
            name=g.bass.get_next_instruction_name(),
            ins=[*_in_ap, _idxs_ap, g.lower_val_access(g.to_reg(num_idxs_reg))],
            outs=[_out_ap],
            transpose=False,
            num_idxs=num_idxs,
            elem_size=elem_size,
            stride_bytes_256=stride_bytes_256,
            gen_mode=0,
            single_packet=False,
            queue_num=queue_num,
            sbuf_tokens_per_rank=0,
            sbuf_free_dim_per_rank=0,
            sbuf_free_dim_pad_per_rank=0,
            sbuf_byte_offset=0,
        )
    )


def _finalize(nc, mybir):
    """Lower extended instructions for the plain-Bass (non-Bacc) pipeline:
    insert GPSIMD library loads, encode InstISA bytes, and split multi-wait
    sync infos (TRN2 ISA allows one sem wait per instruction)."""
    import bass_rust as _bass_rust
    from concourse.library_config import all_libraries, standard

    mask: dict = {}
    for lib in all_libraries:
        for t in lib.instructions:
            mask[t] = mask.get(t, 0) | (1 << lib.index)
    _bass_rust.insert_library_loads(nc, mask, len(all_libraries), standard.index)
    mybir.codegen_inst_isa_subclasses(nc)
    _split_multi_waits(nc, mybir)


def _build():
    import concourse.bass as bass
    import concourse.mybir as mybir
    from concourse.tile import TileContext

    nc = bass.Bass(num_swdge_queues=NQ)
    idx16 = nc.dram_tensor(
        "idx16", [P, IDX_COLS], mybir.dt.int16, kind="ExternalInput"
    )
    dt = nc.dram_tensor(
        "dt", [NCHUNK * MODES * DT_ROWS, DT_W], mybir.dt.float32,
        kind="ExternalInput",
    )
    out = nc.dram_tensor("out", [P, W_OUT], mybir.dt.float32,
                         kind="ExternalOutput")

    with TileContext(nc) as tc:
        with (
            tc.tile_pool(name="io", bufs=1) as io_pool,
            tc.tile_pool(name="gat", bufs=6) as gat_pool,
            tc.tile_pool(name="prd", bufs=2) as prd_pool,
        ):
            nregs = {}
            for sz in sorted(set(SUB_SIZES)):
                r = nc.gpsimd.alloc_register(f"nidx{sz}")
                nc.gpsimd.reg_mov(r, sz)
                nregs[sz] = r
            idx_sb = io_pool.tile([P, IDX_COLS], mybir.dt.int16)
            nc.sync.dma_start(out=idx_sb[:], in_=idx16[:])
            out_sb = io_pool.tile([P, W_OUT], mybir.dt.float32)
            gidx = 0
            col0 = 0
            for t in range(NSUB):
                st = SUB_SIZES[t]
                cols = st // P
                c = SUB_BASE[t] // CHUNK  # owning chunk (sub-chunks align)
                g3 = []
                for m in range(MODES):
                    g = gat_pool.tile([P, (S // P) * RANK],
                                      mybir.dt.float32, tag="g")
                    row0 = (c * MODES + m) * DT_ROWS
                    off = _IDX_OFF[t * MODES + m]
                    dma_gather_raw(
                        nc.gpsimd,
                        out_ap=g[:, :cols * RANK].rearrange(
                            "p (c e) -> p c e", e=RANK
                        ),
                        in_ap=dt[row0:row0 + DT_ROWS, :RANK],
                        idxs_ap=idx_sb[:, off:off + st // 16],
                        num_idxs=st,
                        num_idxs_reg=nregs[st],
                        elem_size=RANK,
                        elem_step=DT_W,
                        queue_num=gidx % NQ,
                    )
                    gidx += 1
                    g3.append(g)
                tmp = prd_pool.tile([P, (S // P) * RANK],
                                    mybir.dt.float32, tag="tmp")
                nc.vector.tensor_mul(
                    out=tmp[:, :cols * RANK],
                    in0=g3[0][:, :cols * RANK],
                    in1=g3[1][:, :cols * RANK],
                )
                nc.vector.tensor_mul(
                    out=tmp[:, :cols * RANK],
                    in0=tmp[:, :cols * RANK],
                    in1=g3[2][:, :cols * RANK],
                )
                nc.vector.reduce_sum(
                    out=out_sb[:, col0:col0 + cols],
                    in_=tmp[:, :cols * RANK].rearrange(
                        "p (c r) -> p c r", r=RANK
                    ),
                    axis=mybir.AxisListType.X,
                )
                col0 += cols
            nc.sync.dma_start(out=out[:], in_=out_sb[:])

    _finalize(nc, mybir)
    return nc


def _split_multi_waits(nc, mybir):
    """The TRN2 ISA embeds at most ONE sem wait per instruction; Tile
    sometimes attaches several.  Hoist the extras into standalone
    EventSemaphore instructions placed immediately before the owner in the
    same block — same engine queue, same order, identical semantics."""
    for blk in nc.m.functions[0].blocks:
        new_insts = []
        for inst in blk.instructions:
            si = inst.sync_info
            if si is not None and si.on_wait and len(si.on_wait) > 1:
                extra, keep = list(si.on_wait[:-1]), [si.on_wait[-1]]
                for j, w in enumerate(extra):
                    new_insts.append(
                        mybir.InstEventSemaphore(
                            name=f"{inst.name}-esw{j}",
                            engine=inst.engine,
                            ins=[],
                            outs=[],
                            sync_info=mybir.SyncInfo(on_wait=[w], on_update=[]),
                        )
                    )
                si.on_wait = keep
            new_insts.append(inst)
        blk.instructions = new_insts


def _get_nc():
    if "nc" not in _cache:
        _cache["nc"] = _build()
    return _cache["nc"]


def _wrap16(inv: np.ndarray) -> np.ndarray:
    """[S] int16 -> [128, S//16]: idx k at partition k%16, column k//16,
    replicated across the 8 groups of 16 partitions (one per Q7 core)."""
    a = inv.reshape(-1, 16).T
    return np.tile(a, (8, 1))


def _prep_in_maps(idxs, f0, f1, f2):
    idx = np.asarray(idxs).astype(np.int32)
    assert idx.shape == (NNZ, MODES), idx.shape
    fs = [np.asarray(f, dtype=np.float32) for f in (f0, f1, f2)]

    sub_base = SUB_BASE
    in_maps = []
    for k in range(N_CORES):
        e = idx[k * N_PER_CORE:(k + 1) * N_PER_CORE]
        ep = np.zeros((N_PAD, MODES), dtype=np.int32)
        ep[:N_PER_CORE] = e
        dt = np.zeros((NCHUNK * MODES * DT_ROWS, DT_W), dtype=np.float32)
        idx16 = np.empty((P, IDX_COLS), dtype=np.int16)
        for c in range(NCHUNK):
            lo, hi = c * CHUNK, min((c + 1) * CHUNK, N_PAD)
            for m in range(MODES):
                a = ep[lo:hi, m]
                uniq, inv = np.unique(a, return_inverse=True)
                r0 = (c * MODES + m) * DT_ROWS
                dt[r0:r0 + len(uniq), :RANK] = fs[m][uniq]
                inv16 = inv.astype(np.int16)
                for t in range(NSUB):
                    if not (lo <= sub_base[t] < hi):
                        continue
                    s0, s1 = sub_base[t] - lo, sub_base[t + 1] - lo
                    off = _IDX_OFF[t * MODES + m]
                    idx16[:, off:off + (s1 - s0) // 16] = _wrap16(
                        inv16[s0:s1]
                    )
        in_maps.append({"idx16": idx16, "dt": dt})
    return in_maps


def run(inputs: dict, trace: bool = False):
    """Run the kernel on 8 cores; returns (full_output, BassKernelResults)."""
    from concourse.bass_utils import run_bass_kernel_spmd

    in_maps = _prep_in_maps(
        inputs["idxs"], inputs["f0"], inputs["f1"], inputs["f2"]
    )
    nc = _get_nc()
    res = run_bass_kernel_spmd(
        nc,
        in_maps,
        core_ids=list(range(N_CORES)),
        trace=trace,
    )
    out = np.concatenate(
        [r["out"].T.reshape(-1)[:N_PER_CORE] for r in res.results]
    )
    return out, res


def kernel(**inputs) -> np.ndarray:
    out, _ = run(inputs, trace=False)
    return out


# revision 6
# speedup vs baseline: 1.1453x; 1.1453x over previous
"""CPD reconstruction at observed entries (embedding-lookup style) on 8 TRN2 cores.

rec[n] = sum_r f0[i0[n],r] * f1[i1[n],r] * f2[i2[n],r]   for n in [0, 1M)

Strategy: data-parallel over the nnz axis across the 8 cores (125k entries
each).  The per-row random gathers use the Anthropic extended DMA-gather
instruction (InstDMAGatherAnt): ONE instruction emits 8192 row descriptors
from an int16 index list, generated by a GPSIMD Q7 core pair; rotating the
4 SWDGE queues runs 4 pairs in parallel.  That replaces the baseline's
per-128-row indirect DMAs (994ns fixed SWDGE cost each, serial on the Pool
engine) and is ~4.5x faster end to end.

dma_gather constraints and how they are met:
  * int16 indices (0..32767): entries are processed in chunks of 32768; the
    host builds a per-(chunk, mode) DEDUP TABLE holding np.unique(rows) of
    that chunk (<= 32768 rows, so the rank of each entry's row always fits
    int16), and remaps indices to ranks.  Gathers stay in entry order - no
    realignment anywhere.
  * the HBM row stride must be a multiple of 256B: dedup tables are padded
    to 64 f32 per row.  The payload per descriptor stays 128B (elem_size=32,
    elem_step=64) - the bass-level elem%256B assert is Q7-side unnecessary,
    so we emit the instruction directly (dma_gather_raw).
  * descriptors are packetized (single_packet=False); the default
    single-packet mode caps a gather at 64 descriptors/lane and crashes the
    exec unit beyond that.

Per sub-chunk of 8192 entries the three modes' gathered rows land as
[128, 64, 32] tiles (entry n at partition n%128, column n//128); DVE
multiplies the three tiles and reduces over rank into the output column
block.  Host un-interleaves at the end.
"""

import numpy as np

NNZ = 1_000_000
RANK = 32
N_CORES = 8
N_PER_CORE = NNZ // N_CORES  # 125_000
P = 128
CHUNK = 32_768  # entries per dedup-table chunk (int16 rank guarantee)
NCHUNK = -(-N_PER_CORE // CHUNK)  # 4
MODES = 3
S = 8_192  # indices per dma_gather; uniform size keeps the 4 Q7 pairs balanced
SUB_SIZES = [S] * 15 + [2_176]
N_PAD = sum(SUB_SIZES)  # 125_056 entries per core after padding
SUB_BASE = [0]
for _s in SUB_SIZES:
    SUB_BASE.append(SUB_BASE[-1] + _s)
NSUB = len(SUB_SIZES)
DT_ROWS = CHUNK  # dedup table rows per (chunk, mode)
DT_W = 64  # f32 per dedup row (256B stride; first 32 hold data)
W_OUT = N_PAD // P  # 977 output columns
NQ = 4  # SWDGE queues
# idx16 column offset of (sub-chunk t, mode m); width SUB_SIZES[t]//16
_IDX_OFF = []
_o = 0
for _t in range(NSUB):
    for _m in range(MODES):
        _IDX_OFF.append(_o)
        _o += SUB_SIZES[_t] // 16
IDX_COLS = _o

_cache: dict = {}


def dma_gather_raw(g, out_ap, in_ap, idxs_ap, num_idxs, num_idxs_reg,
                   elem_size, elem_step, queue_num):
    """bass.dma_gather without the elem_size%256B assert (stride must still be
    a 256B multiple; the Q7 kernel handles any payload size) and with
    packetized descriptors."""
    import concourse.mybir as mybir

    g._assert_queue_num(queue_num)
    assert idxs_ap.dtype == mybir.dt.int16
    assert in_ap.dtype == out_ap.dtype
    stride_bytes = elem_step * mybir.dt.size(in_ap.dtype)
    stride_bytes_256 = stride_bytes // 256
    assert stride_bytes % 256 == 0 and 0 < stride_bytes_256 < 256
    assert in_ap.ap[0][0] == elem_step
    assert in_ap.ap[-1][1] == out_ap.ap[-1][1] == elem_size

    _in_ap = g.lower_ap_dma(in_ap, for_custom_bir_dma=True)
    _idxs_ap = g.lower_ap(idxs_ap)
    _out_ap = g.lower_ap(out_ap)
    return g.add_instruction(
        mybir.InstDMAGatherAnt(
            name=g.bass.get_next_instruction_name(),
            ins=[*_in_ap, _idxs_ap, g.lower_val_access(g.to_reg(num_idxs_reg))],
            outs=[_out_ap],
            transpose=False,
            num_idxs=num_idxs,
            elem_size=elem_size,
            stride_bytes_256=stride_bytes_256,
            gen_mode=0,
            single_packet=False,
            queue_num=queue_num,
            sbuf_tokens_per_rank=0,
            sbuf_free_dim_per_rank=0,
            sbuf_free_dim_pad_per_rank=0,
            sbuf_byte_offset=0,
        )
    )


def _finalize(nc, mybir):
    """Lower extended instructions for the plain-Bass (non-Bacc) pipeline:
    insert GPSIMD library loads, encode InstISA bytes, and split multi-wait
    sync infos (TRN2 ISA allows one sem wait per instruction)."""
    import bass_rust as _bass_rust
    from concourse.library_config import all_libraries, standard

    mask: dict = {}
    for lib in all_libraries:
        for t in lib.instructions:
            mask[t] = mask.get(t, 0) | (1 << lib.index)
    _bass_rust.insert_library_loads(nc, mask, len(all_libraries), standard.index)
    mybir.codegen_inst_isa_subclasses(nc)
    _split_multi_waits(nc, mybir)


def _build():
    import concourse.bass as bass
    import concourse.mybir as mybir
    from concourse.tile import TileContext

    nc = bass.Bass(num_swdge_queues=NQ)
    idx16 = nc.dram_tensor(
        "idx16", [P, IDX_COLS], mybir.dt.int16, kind="ExternalInput"
    )
    dt = nc.dram_tensor(
        "dt", [NCHUNK * MODES * DT_ROWS, DT_W], mybir.dt.float32,
        kind="ExternalInput",
    )
    out = nc.dram_tensor("out", [P, W_OUT], mybir.dt.float32,
                         kind="ExternalOutput")

    with TileContext(nc) as tc:
        with (
            tc.tile_pool(name="io", bufs=1) as io_pool,
            tc.tile_pool(name="gat", bufs=6) as gat_pool,
            tc.tile_pool(name="prd", bufs=3) as prd_pool,
        ):
            nregs = {}
            for sz in sorted(set(SUB_SIZES)):
                r = nc.gpsimd.alloc_register(f"nidx{sz}")
                nc.gpsimd.reg_mov(r, sz)
                nregs[sz] = r
            idx_sb = io_pool.tile([P, IDX_COLS], mybir.dt.int16)
            nc.sync.dma_start(out=idx_sb[:], in_=idx16[:])
            out_sb = io_pool.tile([P, W_OUT], mybir.dt.float32)
            gidx = 0
            col0 = 0
            for t in range(NSUB):
                st = SUB_SIZES[t]
                cols = st // P
                c = SUB_BASE[t] // CHUNK  # owning chunk (sub-chunks align)
                g3 = []
                for m in range(MODES):
                    g = gat_pool.tile([P, (S // P) * RANK],
                                      mybir.dt.float32, tag="g")
                    row0 = (c * MODES + m) * DT_ROWS
                    off = _IDX_OFF[t * MODES + m]
                    dma_gather_raw(
                        nc.gpsimd,
                        out_ap=g[:, :cols * RANK].rearrange(
                            "p (c e) -> p c e", e=RANK
                        ),
                        in_ap=dt[row0:row0 + DT_ROWS, :RANK],
                        idxs_ap=idx_sb[:, off:off + st // 16],
                        num_idxs=st,
                        num_idxs_reg=nregs[st],
                        elem_size=RANK,
                        elem_step=DT_W,
                        queue_num=gidx % NQ,
                    )
                    gidx += 1
                    g3.append(g)
                tmp = prd_pool.tile([P, (S // P) * RANK],
                                    mybir.dt.float32, tag="tmp")
                nc.vector.tensor_mul(
                    out=tmp[:, :cols * RANK],
                    in0=g3[0][:, :cols * RANK],
                    in1=g3[1][:, :cols * RANK],
                )
                nc.vector.tensor_mul(
                    out=tmp[:, :cols * RANK],
                    in0=tmp[:, :cols * RANK],
                    in1=g3[2][:, :cols * RANK],
                )
                nc.vector.reduce_sum(
                    out=out_sb[:, col0:col0 + cols],
                    in_=tmp[:, :cols * RANK].rearrange(
                        "p (c r) -> p c r", r=RANK
                    ),
                    axis=mybir.AxisListType.X,
                )
                col0 += cols
            nc.sync.dma_start(out=out[:], in_=out_sb[:])

    _finalize(nc, mybir)
    return nc


def _split_multi_waits(nc, mybir):
    """The TRN2 ISA embeds at most ONE sem wait per instruction; Tile
    sometimes attaches several.  Hoist the extras into standalone
    EventSemaphore instructions placed immediately before the owner in the
    same block — same engine queue, same order, identical semantics."""
    for blk in nc.m.functions[0].blocks:
        new_insts = []
        for inst in blk.instructions:
            si = inst.sync_info
            if si is not None and si.on_wait and len(si.on_wait) > 1:
                extra, keep = list(si.on_wait[:-1]), [si.on_wait[-1]]
                for j, w in enumerate(extra):
                    new_insts.append(
                        mybir.InstEventSemaphore(
                            name=f"{inst.name}-esw{j}",
                            engine=inst.engine,
                            ins=[],
                            outs=[],
                            sync_info=mybir.SyncInfo(on_wait=[w], on_update=[]),
                        )
                    )
                si.on_wait = keep
            new_insts.append(inst)
        blk.instructions = new_insts


def _get_nc():
    if "nc" not in _cache:
        _cache["nc"] = _build()
    return _cache["nc"]


def _wrap16(inv: np.ndarray) -> np.ndarray:
    """[S] int16 -> [128, S//16]: idx k at partition k%16, column k//16,
    replicated across the 8 groups of 16 partitions (one per Q7 core)."""
    a = inv.reshape(-1, 16).T
    return np.tile(a, (8, 1))


def _prep_in_maps(idxs, f0, f1, f2):
    idx = np.asarray(idxs).astype(np.int32)
    assert idx.shape == (NNZ, MODES), idx.shape
    fs = [np.asarray(f, dtype=np.float32) for f in (f0, f1, f2)]

    sub_base = SUB_BASE
    in_maps = []
    for k in range(N_CORES):
        e = idx[k * N_PER_CORE:(k + 1) * N_PER_CORE]
        ep = np.zeros((N_PAD, MODES), dtype=np.int32)
        ep[:N_PER_CORE] = e
        dt = np.zeros((NCHUNK * MODES * DT_ROWS, DT_W), dtype=np.float32)
        idx16 = np.empty((P, IDX_COLS), dtype=np.int16)
        for c in range(NCHUNK):
            lo, hi = c * CHUNK, min((c + 1) * CHUNK, N_PAD)
            for m in range(MODES):
                a = ep[lo:hi, m]
                uniq, inv = np.unique(a, return_inverse=True)
                r0 = (c * MODES + m) * DT_ROWS
                dt[r0:r0 + len(uniq), :RANK] = fs[m][uniq]
                inv16 = inv.astype(np.int16)
                for t in range(NSUB):
                    if not (lo <= sub_base[t] < hi):
                        continue
                    s0, s1 = sub_base[t] - lo, sub_base[t + 1] - lo
                    off = _IDX_OFF[t * MODES + m]
                    idx16[:, off:off + (s1 - s0) // 16] = _wrap16(
                        inv16[s0:s1]
                    )
        in_maps.append({"idx16": idx16, "dt": dt})
    return in_maps


def run(inputs: dict, trace: bool = False):
    """Run the kernel on 8 cores; returns (full_output, BassKernelResults)."""
    from concourse.bass_utils import run_bass_kernel_spmd

    in_maps = _prep_in_maps(
        inputs["idxs"], inputs["f0"], inputs["f1"], inputs["f2"]
    )
    nc = _get_nc()
    res = run_bass_kernel_spmd(
        nc,
        in_maps,
        core_ids=list(range(N_CORES)),
        trace=trace,
    )
    out = np.concatenate(
        [r["out"].T.reshape(-1)[:N_PER_CORE] for r in res.results]
    )
    return out, res


def kernel(**inputs) -> np.ndarray:
    out, _ = run(inputs, trace=False)
    return out
